# revision 63
# baseline (speedup 1.0000x reference)
"""Trainium2 Bass kernel for nn_MultiHeadAttn (B=4, NQ=NK=2048, D=1024, H=8).

Sharding: 8 cores = 4 batches x 2 query-halves. Each core owns 1024 query rows
of one batch; k/v projections for that batch are computed redundantly by the
two cores sharing it (cheap after key compaction + fp8).

Key compaction: the mask is host-visible and ~50% of keys are masked
(their attention weight is exactly 0), so the host gathers the unmasked
keys per batch and pads to KCAP (multiple of 128). This halves kproj,
vproj, logits, A*V, den and the exp volume.

Precision: the attention branch is strongly attenuated in the output
(softmax over ~1K near-uniform keys -> att is ~3% of the residual qp), so
it runs in fp8e4m3 with DoubleRow matmuls: k, v, Wk*16, Wv*16, vp*16 and
exp(logits) are fp8. The residual path (qproj, MLP, layernorms) runs in
bf16 activations with f32 PSUM accumulation.

Engine balance (per core): PE issue slots are ~213ns per 512-col matmul
at 2.4GHz; scalar-engine EXP costs ~0.78ns/elem + ~320ns fixed per ACT.
The kernel runs ONE merged projection+attention pipeline: qproj first
(exps need qp and kp), then the lg->exp->att/den pipeline starts with
kproj h0 + the vp half for heads 0-3 done, and the remaining kproj/vproj
chunks stream into the PE as filler work between logits groups while the
scalar engine grinds exps — the PE never idles waiting for exp and the
scalar engine never idles waiting for projections. den uses fp8 DR pairs
into PSUM rows 0:32 with mones=16 folding the scale; its reciprocal is
partition-broadcast on the otherwise-idle GPSIMD. Attention drains also
pre-compute x1^2 (fp8) so LN1 stats matmuls start immediately. LN math
runs bf16 on the DVE (squares/casts on ACT), E[x^2] via fp8 DR pairs
(onesn8 = 2^-9 subnormal), the g/b affine via tensor_scalar — skipped
entirely when the host detects g=1,b=0. Output is stored bf16 and
upcast on the host.

Per-core dataflow (activations feature-major "T layout" [feat, row]):
  qpT = Wq @ qT            (bf16, dt-outer, double-buffered PSUM)
  vp  = v @ Wv.T * 16      (fp8 DoubleRow, natural [key, feat] layout)
  kpT = (Wk*16) @ kT       (fp8 DoubleRow, bf16 out at 16x scale)
  attention, 80 pipelined (head, chunk, key-tile-pair) steps:
      logitsT[kk,r] = kpT_h_tile.T @ xbfT_h   (bf16 matmul, PSUM f32)
      expT = Exp(logitsT / 512)               (ACT per pair, fp8 out)
      attT += vp_pair.T @ expT                (fp8 DoubleRow accumulate)
      denT += (16*mones_pair).T @ expT        (fp8 DoubleRow, rows 0:32)
  x1T = qpT + attT * recip(denT)   (gpsimd partition-broadcast of recip)
  out1 = LN(x1) [* g1 + b1]   via ones-matmul stats, bf16 DVE normalize
  x2T = out1 + Relu(Wout @ out1T + bout)      (bf16 matmul, ACT bias+relu)
  outT = LN(x2) [* g2 + b2] -> DRAM bf16 [P, feat-tile, row]; host
  reassembles and upcasts.
"""

from contextlib import ExitStack

import numpy as np
import ml_dtypes

import concourse.mybir as mybir
import concourse.tile as tile
from concourse import bacc
from concourse.bass_utils import run_bass_kernel_spmd

B, NQ, NK, D, H = 4, 2048, 2048, 1024, 8
DH = D // H            # 128, head dim
P = 128                # partitions
RQ = NQ // 2           # 1024 query rows per core
EPS = 1e-5

F32 = mybir.dt.float32
BF16 = mybir.dt.bfloat16
FP8 = mybir.dt.float8e4
BFNP = ml_dtypes.bfloat16
F8NP = ml_dtypes.float8_e4m3

KT = D // P            # 8 contraction tiles over features
DT = D // P            # 8 output-feature tiles (also heads)
RC = RQ // 512         # 2 row chunks of 512
DR = mybir.MatmulPerfMode.DoubleRow


def build_nc(kcap, plain_ln=False, debug=False):
    """kcap: padded (compacted) key count, multiple of 128.
    plain_ln: host-detected g1=g2=1, b1=b2=0 (skips the LN affines)."""
    assert kcap % 128 == 0
    KKT = kcap // 128          # key tiles
    KKTP = 2 * ((KKT + 1) // 2)    # mones key tiles padded to even
    kchunks = []
    o = 0
    while o < kcap:            # kproj output chunks (N dim), each <= 512
        n = min(512, kcap - o)
        kchunks.append((o, n))
        o += n

    # exp groups: key-tile pairs (DR-pair aligned for att/den); the scalar
    # engine is no longer the bottleneck once projections overlap attention
    GS = []
    t0 = 0
    while t0 < KKT:
        GS.append((t0, min(2, KKT - t0)))
        t0 += 2
    NG = len(GS)

    nc = bacc.Bacc("TRN2", target_bir_lowering=False, debug=debug)

    # all inputs pre-arranged to [P, tile, cols] on the host
    qT = nc.declare_dram_parameter("qT", [P, KT, RQ], BF16, isOutput=False)
    kT = nc.declare_dram_parameter("kT", [P, KT, kcap], FP8, isOutput=False)
    vT = nc.declare_dram_parameter("vT", [P, KT, kcap], FP8, isOutput=False)
    wqT = nc.declare_dram_parameter("wqT", [P, KT, D], BF16, isOutput=False)
    wkT = nc.declare_dram_parameter("wkT", [P, KT, D], FP8, isOutput=False)
    wvT = nc.declare_dram_parameter("wvT", [P, KT, D], FP8, isOutput=False)
    woT = nc.declare_dram_parameter("woT", [P, KT, D], BF16, isOutput=False)
    mones = nc.declare_dram_parameter("mones", [P, KKTP * 32], FP8, isOutput=False)
    vecs = nc.declare_dram_parameter("vecs", [P, 5, DT], F32, isOutput=False)
    outT = nc.declare_dram_parameter("outT", [P, DT, RQ], BF16, isOutput=True)

    Act = mybir.ActivationFunctionType

    with tile.TileContext(nc) as tc, ExitStack() as ctx:
        consts = ctx.enter_context(tc.tile_pool(name="consts", bufs=1))
        pool_qp = ctx.enter_context(tc.tile_pool(name="pool_qp", bufs=1))

        onesn = consts.tile([P, P], BF16)
        nc.vector.memset(onesn, 1.0 / D)
        # fp8 DR stationary for the E[x^2] matmuls: 2^-9 is the smallest
        # fp8e4 subnormal, so the sum over 1024 features is 2*E[x^2] and
        # the var chain halves it back
        onesn8 = consts.tile([P, 2, P], FP8)
        nc.vector.memset(onesn8, 2.0 ** -9)
        eps_sb = consts.tile([P, 1], F32)
        nc.vector.memset(eps_sb, EPS)
        # PE warm-up source: the DVFS ramp needs ~3us of continuous matmul
        # execution before reaching max clock, so dummy matmuls run during
        # the initial DMA wait (values irrelevant, result never read)
        warm_src = consts.tile([P, 512], BF16)
        nc.vector.memset(warm_src, 0.0)

        # persistent activations: qp -> x1 -> x2, bf16 (~0.4% rounding is
        # well inside the error budget and doubles DVE throughput)
        xbf_sb = pool_qp.tile([P, DT, RQ], BF16)
        # x1^2 (fp8), filled by the attention drains so LN1's E[x^2]
        # matmuls start immediately after the last drain
        sq1_sb = pool_qp.tile([P, DT, RQ], FP8)


        with (
            tc.tile_pool(name="pool_attn", bufs=1) as pool_attn,
            tc.tile_pool(name="pool_ain", bufs=1) as ain,
        ):
            kpT_sb = pool_attn.tile([P, H, kcap], BF16)  # per-head [dh, key], 16x
            vp_sb = pool_attn.tile([P, KKT, D], FP8)     # per key-tile [key, feat], 16x
            # ------- Merged pipeline: projections + attention -------
            # qproj runs first (exps need qp and kp), then the attention
            # lg->exp->att/den pipeline starts with kproj h0 + the vp half
            # used by heads 0-3 done; the REMAINING kproj heads and vp half
            # stream into the PE as filler work between logits groups while
            # the scalar engine grinds exps. All retained keys are unmasked;
            # zero-padded tail keys are excluded via zeroed vp rows and
            # zeroed den lhsT (mones, value 16 so den_ps = 16*den).
            with (
                tc.tile_pool(name="a_ps", bufs=2, space="PSUM") as a_ps,
                tc.tile_pool(name="att_ps", bufs=1, space="PSUM") as att_psp,
                tc.tile_pool(name="den_ps", bufs=1, space="PSUM") as den_psp,
                tc.tile_pool(name="lg_ps", bufs=2, space="PSUM") as lg_psp,
                tc.tile_pool(name="bsb", bufs=1) as bsb,
            ):
                # PE warm-up during the initial DMA wait (see warm_src)
                warm_ps = a_ps.tile([P, 512], F32, tag="aps")
                for _ in range(13):
                    nc.tensor.matmul(warm_ps, onesn, warm_src,
                                     start=True, stop=True)

                wq_sb = ain.tile([P, KT, D], BF16, tag="wq")
                qT_sb = ain.tile([P, KT, RQ], BF16, tag="qt")
                # kt-tile DMAs in first-use order: singles first so the
                # opening matmuls are gated on the smallest transfers
                for t0_, tn in ((0, 1), (1, 1), (2, 2), (4, 2), (6, 2)):
                    nc.sync.dma_start(out=wq_sb[:, t0_:t0_ + tn, :],
                                      in_=wqT[:, t0_:t0_ + tn, :])
                    nc.sync.dma_start(out=qT_sb[:, t0_:t0_ + tn, 0:512],
                                      in_=qT[:, t0_:t0_ + tn, 0:512])
                for t4 in range(0, KT, 4):
                    nc.sync.dma_start(out=qT_sb[:, t4:t4 + 4, 512:1024],
                                      in_=qT[:, t4:t4 + 4, 512:1024])
                kT_sb = ain.tile([P, KT, kcap], FP8, tag="kt")
                nc.sync.dma_start(out=kT_sb, in_=kT[:, :, :])
                wkT_sb = ain.tile([P, KT, D], FP8, tag="wk")
                nc.sync.dma_start(out=wkT_sb, in_=wkT[:, :, :])
                vT_sb = ain.tile([P, KT, kcap], FP8, tag="vv")
                nc.sync.dma_start(out=vT_sb, in_=vT[:, :, :])
                wvT_sb = ain.tile([P, KT, D], FP8, tag="wv")
                nc.sync.dma_start(out=wvT_sb, in_=wvT[:, :, :])
                mones_sb = consts.tile([P, KKTP, 32], FP8)
                nc.sync.dma_start(out=mones_sb, in_=mones[:, :])
                vecs_sb = consts.tile([P, 5, DT], F32)
                nc.sync.dma_start(out=vecs_sb, in_=vecs[:, :, :])
                g1_sb, b1_sb, g2_sb, b2_sb, bo_sb = (
                    vecs_sb[:, i, :] for i in range(5))

                # q projection, dt-outer (a_ps double-buffered; the first
                # matmul needs only the first wq/qT kt DMAs)
                for c in range(RC):
                    for dt_ in range(DT):
                        ps = a_ps.tile([P, 512], F32, tag="aps")
                        for kt in range(KT):
                            nc.tensor.matmul(
                                ps,
                                wq_sb[:, kt, dt_ * P:(dt_ + 1) * P],
                                qT_sb[:, kt, c * 512:(c + 1) * 512],
                                start=(kt == 0), stop=(kt == KT - 1),
                            )
                        nc.vector.tensor_copy(
                            xbf_sb[:, dt_, c * 512:(c + 1) * 512], ps)

                def emit_kproj(h, co, cn):
                    # one kproj chunk: kpT[h, co:co+cn] = ((Wk*16) @ k.T)
                    ps = a_ps.tile([P, 512], F32, tag="aps")
                    for tp in range(KT // 2):
                        nc.tensor.matmul(
                            ps[:, 0:cn],
                            wkT_sb[:, 2 * tp:2 * tp + 2, h * P:(h + 1) * P],
                            kT_sb[:, 2 * tp:2 * tp + 2, co:co + cn],
                            start=(tp == 0), stop=(tp == KT // 2 - 1),
                            perf_mode=DR,
                        )
                    nc.vector.tensor_copy(kpT_sb[:, h, co:co + cn],
                                          ps[:, 0:cn])

                def emit_vproj(kkt, dh):
                    # one vproj chunk: vp[kkt-tile, dh*512:...] * 16
                    ps = a_ps.tile([P, 512], F32, tag="aps")
                    for tp in range(KT // 2):
                        nc.tensor.matmul(
                            ps,
                            vT_sb[:, 2 * tp:2 * tp + 2, kkt * P:(kkt + 1) * P],
                            wvT_sb[:, 2 * tp:2 * tp + 2, dh * 512:(dh + 1) * 512],
                            start=(tp == 0), stop=(tp == KT // 2 - 1),
                            perf_mode=DR,
                        )
                    nc.vector.tensor_copy(vp_sb[:, kkt, dh * 512:(dh + 1) * 512],
                                          ps)

                # pre-loop: kproj head 0 and the vp D-half used by heads 0-3
                for (co, cn) in kchunks:
                    emit_kproj(0, co, cn)
                for kkt in range(KKT):
                    emit_vproj(kkt, 0)

                # filler units for the attention loop, in deadline order:
                # kproj for head h must land before head h's logits, the
                # second vp half before head 4's att matmuls
                fillers = []
                vp1 = [(kkt, 1) for kkt in range(KKT)]
                n3 = (len(vp1) + 2) // 3
                for hh in range(1, H):
                    fillers += [("kp", hh, co, cn) for (co, cn) in kchunks]
                    if hh <= 3:
                        tk = vp1[(hh - 1) * n3:hh * n3]
                        fillers += [("vp", kkt, dh, None) for (kkt, dh) in tk]

                steps = [(h, c, j)
                         for h in range(H) for c in range(RC) for j in range(NG)]
                exq = {}      # (h, c, j) -> ex tile
                cur = {}      # (h, c) -> (att_ps, den_ps)

                def emit_drain(h, c, att_ps, den_ps):
                    rs = slice(c * 512, (c + 1) * 512)
                    # copy att out of PSUM first: the next iteration's att
                    # accumulation reuses the bank, so freeing it fast keeps
                    # the in-order PE stream from stalling on this chain
                    attv = bsb.tile([P, 512], BF16, tag="attv", bufs=2)
                    nc.vector.tensor_copy(attv, att_ps)
                    # den_ps rows 0:32 hold 32 replicas of 16*den
                    rec32 = bsb.tile([32, 512], F32, tag="rec32", bufs=2)
                    nc.vector.reciprocal_approx_fast(rec32, den_ps[0:32, :])
                    rec = bsb.tile([P, 512], F32, tag="rec", bufs=2)
                    nc.gpsimd.partition_broadcast(rec, rec32[0:1, :])
                    att_n = bsb.tile([P, 512], F32, tag="attn", bufs=2)
                    nc.vector.tensor_mul(att_n, attv, rec)  # att/(16 den)
                    # x1 = qp + att  (in place over xbf)
                    nc.vector.tensor_add(xbf_sb[:, h, rs], xbf_sb[:, h, rs],
                                         att_n)
                    # x1^2 for LN1 stats while the DVE has slack
                    nc.vector.tensor_mul(sq1_sb[:, h, rs], xbf_sb[:, h, rs],
                                         xbf_sb[:, h, rs])

                def emit_lgexp(h, c, j):
                    rs = slice(c * 512, (c + 1) * 512)
                    g0, gn = GS[j]
                    lg_ps = lg_psp.tile([P, 2, 512], F32, tag="lg")
                    for t in range(gn):
                        nc.tensor.matmul(
                            lg_ps[:, t, :],
                            kpT_sb[:, h, (g0 + t) * P:(g0 + t + 1) * P],
                            xbf_sb[:, h, rs],
                            start=True, stop=True,
                        )
                    ex = bsb.tile([P, 2, 512], FP8, tag="ex", bufs=NG + 2)
                    # /512 = /16 (kp scale) /32 (sqrt(D))
                    nc.scalar.activation(ex[:, 0:gn, :], lg_ps[:, 0:gn, :],
                                         Act.Exp, scale=1.0 / 512.0)
                    exq[(h, c, j)] = ex

                def emit_attden(h, c, j):
                    if j == 0:
                        cur[(h, c)] = (
                            att_psp.tile([P, 512], F32, tag="att",
                                         name=f"att_{h}_{c}"),
                            den_psp.tile([P, 512], F32, tag="den",
                                         name=f"den_{h}_{c}"),
                        )
                    att_ps, den_ps = cur[(h, c)]
                    ex = exq.pop((h, c, j))
                    # one DR pair (or plain fp8 odd tail) each for att/den;
                    # single PSUM accumulation group per (h, c)
                    g0, gn = GS[j]
                    first = g0 == 0
                    last = g0 + gn == KKT
                    if gn == 2:
                        nc.tensor.matmul(
                            att_ps,
                            vp_sb[:, g0:g0 + 2, h * DH:(h + 1) * DH],
                            ex,
                            start=first, stop=last, perf_mode=DR,
                        )
                        nc.tensor.matmul(
                            den_ps[0:32, :],
                            mones_sb[:, g0:g0 + 2, :],
                            ex,
                            start=first, stop=last, perf_mode=DR,
                            skip_group_check=True,
                        )
                    else:
                        nc.tensor.matmul(
                            att_ps,
                            vp_sb[:, g0, h * DH:(h + 1) * DH],
                            ex[:, 0, :],
                            start=first, stop=last,
                        )
                        nc.tensor.matmul(
                            den_ps[0:32, :],
                            mones_sb[:, g0, :],
                            ex[:, 0, :],
                            start=first, stop=last,
                            skip_group_check=True,
                        )
                    if j == NG - 1:
                        emit_drain(h, c, att_ps, den_ps)
                        del cur[(h, c)]

                fi = 0
                for s in range(len(steps) + 2):
                    # one filler every other step spreads the projection
                    # slots across the whole loop (all deadlines still met)
                    if fi < len(fillers) and s % 2 == 0:
                        f = fillers[fi]
                        fi += 1
                        if f[0] == "kp":
                            emit_kproj(f[1], f[2], f[3])
                        else:
                            emit_vproj(f[1], f[2])
                    if s < len(steps):
                        emit_lgexp(*steps[s])
                    if s >= 2:
                        emit_attden(*steps[s - 2])



        # ---------------- Phase C/D: LN1, MLP, LN2 ----------------
        # DVE does squares, normalize and the g/b affine (tensor_scalar with
        # per-partition AP scalars) on bf16 SBUF operands (fast DVE modes);
        # the scalar engine keeps only relu, mean^2 and sqrt.
        with (
            tc.tile_pool(name="late", bufs=1) as late,
            tc.tile_pool(name="csb", bufs=1) as csb,
            tc.tile_pool(name="c_ps", bufs=2, space="PSUM") as c_ps,
        ):
            woT_sb = late.tile([P, KT, D], BF16)
            nc.sync.dma_start(out=woT_sb, in_=woT[:, :, :])
            x1n_sb = late.tile([P, DT, RQ], BF16)
            rl_sb = late.tile([P, DT, RQ], BF16)
            ot_sb = late.tile([P, DT, RQ], BF16)

            cchunks = [(0, 512), (512, 512)]
            MUL = mybir.AluOpType.mult
            ADD = mybir.AluOpType.add

            def ln_stats_rc(rs, presq=None):
                """stats matmuls for one row-chunk of xbf: mean and E[x^2]
                PSUMs [P,512] (row vectors replicated across partitions).
                presq: pre-computed fp8 x^2 buffer (LN1); otherwise squares
                are computed here split between the scalar engine and DVE."""
                mean_ps = c_ps.tile([P, 512], F32, tag="mean")
                for kt in range(KT):
                    nc.tensor.matmul(
                        mean_ps, onesn, xbf_sb[:, kt, rs],
                        start=(kt == 0), stop=(kt == KT - 1),
                    )
                msq_ps = c_ps.tile([P, 512], F32, tag="msq")
                for pr in range(KT // 2):
                    if presq is not None:
                        sqp = presq[:, 2 * pr:2 * pr + 2, rs]
                    else:
                        sqt = csb.tile([P, 2, 512], FP8, tag="sqp", bufs=3)
                        nc.scalar.square(sqt[:, 0, :], xbf_sb[:, 2 * pr, rs])
                        nc.scalar.square(sqt[:, 1, :],
                                         xbf_sb[:, 2 * pr + 1, rs])
                        sqp = sqt
                    nc.tensor.matmul(
                        msq_ps, onesn8, sqp,
                        start=(pr == 0), stop=(pr == KT // 2 - 1),
                        perf_mode=DR,
                    )
                return mean_ps, msq_ps

            def ln_finish(mean_ps, msq_ps, mean_bf, rsg_bf, rs):
                """var/rstd chain; writes bf16 mean/rstd column slices of
                the full-width [P,RQ] stat tiles. The mean downcast goes
                first: mean_ps is ready before msq_ps, and the normalize
                subs only need the mean."""
                nc.scalar.copy(mean_bf[:, rs], mean_ps)
                musq = csb.tile([P, 512], F32, tag="musq", bufs=2)
                nc.scalar.square(musq, mean_ps)
                var = csb.tile([P, 512], F32, tag="var", bufs=2)
                # msq_ps holds 2*E[x^2] (see onesn8)
                nc.vector.scalar_tensor_tensor(
                    var, msq_ps, 0.5, musq,
                    mybir.AluOpType.mult, mybir.AluOpType.subtract)
                std = csb.tile([P, 512], F32, tag="std", bufs=2)
                nc.scalar.activation(std, var, Act.Sqrt,
                                     bias=eps_sb[:, :], scale=1.0)
                rsg = csb.tile([P, 512], F32, tag="rsg", bufs=2)
                nc.vector.reciprocal_approx_fast(rsg, std)
                nc.scalar.copy(rsg_bf[:, rs], rsg)

            # LN1: x1n = LN(x1) * g1 + b1; stats for both chunks first so
            # the var/rstd latency chains hide under the PE stats matmuls;
            # normalize runs chunk 0 first so the MLP z matmuls start early
            mean1_bf = csb.tile([P, RQ], BF16, tag="mean1b", bufs=1)
            rsg1_bf = csb.tile([P, RQ], BF16, tag="rsg1b", bufs=1)
            stats1 = [ln_stats_rc(slice(ro, ro + rn), presq=sq1_sb)
                      for (ro, rn) in cchunks]
            for ci, (ro, rn) in enumerate(cchunks):
                ln_finish(*stats1[ci], mean1_bf, rsg1_bf, slice(ro, ro + rn))
            for ci, (ro, rn) in enumerate(cchunks):
                rs = slice(ro, ro + rn)
                for kt in range(DT):
                    xc = csb.tile([P, 512], BF16, tag="xc1", bufs=4)
                    nc.vector.tensor_sub(xc, xbf_sb[:, kt, rs],
                                         mean1_bf[:, rs])
                    if plain_ln:
                        nc.vector.tensor_mul(x1n_sb[:, kt, rs], xc,
                                             rsg1_bf[:, rs])
                    else:
                        xh = csb.tile([P, 512], BF16, tag="xh1", bufs=3)
                        nc.vector.tensor_mul(xh, xc, rsg1_bf[:, rs])
                        nc.vector.tensor_scalar(
                            x1n_sb[:, kt, rs], xh, g1_sb[:, kt:kt + 1],
                            b1_sb[:, kt:kt + 1], MUL, ADD)

            # MLP: x2 = x1n + relu(Wout @ x1n.T + bout)  (x2 overwrites xbf);
            # LN2 stats per chunk right after its x2 so chain latency hides
            # under the other chunk's z matmuls
            def mlp_chunk(ro, rn):
                rs = slice(ro, ro + rn)
                for dt_ in range(DT):
                    z_ps = c_ps.tile([P, 512], F32, tag="z")
                    for kt in range(KT):
                        nc.tensor.matmul(
                            z_ps,
                            woT_sb[:, kt, dt_ * P:(dt_ + 1) * P],
                            x1n_sb[:, kt, rs],
                            start=(kt == 0), stop=(kt == KT - 1),
                        )
                    nc.scalar.activation(
                        rl_sb[:, dt_, rs], z_ps, Act.Relu,
                        bias=bo_sb[:, dt_:dt_ + 1], scale=1.0,
                    )
                    nc.vector.tensor_add(xbf_sb[:, dt_, rs],
                                         x1n_sb[:, dt_, rs],
                                         rl_sb[:, dt_, rs])

            mean2_bf = csb.tile([P, RQ], BF16, tag="mean2b", bufs=1)
            rsg2_bf = csb.tile([P, RQ], BF16, tag="rsg2b", bufs=1)

            def norm2_chunk(ro, rn, ci):
                # chunked LN2 normalize + store (bf16 out, host upcasts);
                # chunk 0 hides under chunk 1's stats, so its subs can use
                # gpsimd; the tail chunk splits store issues across queues
                rs = slice(ro, ro + rn)
                for kt in range(DT):
                    xc = csb.tile([P, 512], BF16, tag="xc2", bufs=4)
                    nc.vector.tensor_sub(xc, xbf_sb[:, kt, rs],
                                         mean2_bf[:, rs])
                    if plain_ln:
                        nc.vector.tensor_mul(ot_sb[:, kt, rs], xc,
                                             rsg2_bf[:, rs])
                    else:
                        xh = csb.tile([P, 512], BF16, tag="xh2", bufs=3)
                        nc.vector.tensor_mul(xh, xc, rsg2_bf[:, rs])
                        nc.vector.tensor_scalar(
                            ot_sb[:, kt, rs], xh, g2_sb[:, kt:kt + 1],
                            b2_sb[:, kt:kt + 1], MUL, ADD)
                    nc.sync.dma_start(out=outT[:, kt, rs],
                                      in_=ot_sb[:, kt, rs])

            # chunk-0 normalize runs under chunk-1's stats matmuls so the
            # final tail is only chunk 1's chain + normalize
            mlp_chunk(*cchunks[0])
            stats2_0 = ln_stats_rc(slice(0, 512))
            mlp_chunk(*cchunks[1])
            ln_finish(*stats2_0, mean2_bf, rsg2_bf, slice(0, 512))
            stats2_1 = ln_stats_rc(slice(512, 1024))
            norm2_chunk(*cchunks[0], 0)
            ln_finish(*stats2_1, mean2_bf, rsg2_bf, slice(512, 1024))
            norm2_chunk(*cchunks[1], 1)

    nc.compile()
    return nc


_NC_CACHE = {}


def get_nc(kcap=1280, plain_ln=False, debug=False):
    key = (kcap, plain_ln, debug)
    if key not in _NC_CACHE:
        _NC_CACHE[key] = build_nc(kcap, plain_ln=plain_ln, debug=debug)
    return _NC_CACHE[key]


def choose_plain_ln(g1, b1, g2, b2):
    return bool(
        np.all(np.asarray(g1) == 1.0) and np.all(np.asarray(b1) == 0.0)
        and np.all(np.asarray(g2) == 1.0) and np.all(np.asarray(b2) == 0.0))


def choose_kcap(mask):
    nkeep = int((~np.asarray(mask)).sum(axis=1).max())
    return max(256, -(-nkeep // 128) * 128)


def _tiles(a, cols):
    """[D, cols] -> [P, D//P, cols] partition-tiled layout."""
    return np.ascontiguousarray(
        np.asarray(a).reshape(KT, P, cols).transpose(1, 0, 2))


def shard_inputs(q, k, v, mask, Wq, Wk, Wv, Wout, bout, g1, b1, g2, b2,
                 kcap=None):
    q = np.asarray(q, dtype=np.float32)
    k = np.asarray(k, dtype=np.float32)
    v = np.asarray(v, dtype=np.float32)
    mask = np.asarray(mask)
    if kcap is None:
        kcap = choose_kcap(mask)
    KKT = kcap // 128
    KKTP = 2 * ((KKT + 1) // 2)    # mones tiles padded to even
    bfc = lambda a: np.ascontiguousarray(np.asarray(a, dtype=np.float32)).astype(BFNP)
    f8c = lambda a: np.ascontiguousarray(np.asarray(a, dtype=np.float32)).astype(F8NP)
    vec = lambda a: np.asarray(a, dtype=np.float32).reshape(DT, P).T

    vecs = np.stack([vec(g1), vec(b1), vec(g2), vec(b2), vec(bout)], axis=1)
    shared = {
        "wqT": bfc(_tiles(np.asarray(Wq, np.float32).T, D)),
        "wkT": f8c(_tiles(np.asarray(Wk, np.float32).T * 16.0, D)),
        "wvT": f8c(_tiles(np.asarray(Wv, np.float32).T * 16.0, D)),
        "woT": bfc(_tiles(np.asarray(Wout, np.float32).T, D)),
        "vecs": np.ascontiguousarray(vecs),
    }
    in_maps = []
    for bi in range(B):
        keep = np.where(~mask[bi])[0]
        nk = len(keep)
        kc = np.zeros((D, kcap), np.float32)
        vc = np.zeros((D, kcap), np.float32)
        kc[:, :nk] = k[bi][keep].T
        vc[:, :nk] = v[bi][keep].T
        mo = np.zeros((KKTP * P, 32), np.float32)  # [key, 32] -> [P, KKTP*32]
        mo[:nk] = 16.0    # den_ps = 16*den; its reciprocal undoes vp*16 too
        mo = mo.reshape(KKTP, P, 32).transpose(1, 0, 2).reshape(P, KKTP * 32)
        per_batch = {
            "kT": f8c(_tiles(kc, kcap)),
            "vT": f8c(_tiles(vc, kcap)),
            "mones": f8c(mo),
            **shared,
        }
        for half in range(2):
            rows = slice(half * RQ, (half + 1) * RQ)
            in_maps.append({
                "qT": bfc(_tiles(q[bi, rows].T, RQ)),
                **per_batch,
            })
    return in_maps


def assemble_output(results):
    out = np.empty((B, NQ, D), dtype=np.float32)
    for c in range(8):
        bi, half = divmod(c, 2)
        rows = slice(half * RQ, (half + 1) * RQ)
        # outT [P, DT, RQ] bf16 -> [RQ, DT*P] f32
        o = np.asarray(results[c]["outT"]).astype(np.float32)
        out[bi, rows, :] = o.transpose(2, 1, 0).reshape(RQ, D)
    return out


def kernel(**inputs):
    kcap = choose_kcap(inputs["mask"])
    plain = choose_plain_ln(inputs["g1"], inputs["b1"],
                            inputs["g2"], inputs["b2"])
    nc = get_nc(kcap, plain_ln=plain)
    in_maps = shard_inputs(**inputs, kcap=kcap)
    res = run_bass_kernel_spmd(nc, in_maps, core_ids=list(range(8)))
    return assemble_output(res.results)


# revision 64
# speedup vs baseline: 1.0028x; 1.0028x over previous
"""Trainium2 Bass kernel for nn_MultiHeadAttn (B=4, NQ=NK=2048, D=1024, H=8).

Sharding: 8 cores = 4 batches x 2 query-halves. Each core owns 1024 query rows
of one batch; k/v projections for that batch are computed redundantly by the
two cores sharing it (cheap after key compaction + fp8).

Key compaction: the mask is host-visible and ~50% of keys are masked
(their attention weight is exactly 0), so the host gathers the unmasked
keys per batch and pads to KCAP (multiple of 128). This halves kproj,
vproj, logits, A*V, den and the exp volume.

Precision: the attention branch is strongly attenuated in the output
(softmax over ~1K near-uniform keys -> att is ~3% of the residual qp), so
it runs in fp8e4m3 with DoubleRow matmuls: k, v, Wk*16, Wv*16, vp*16 and
exp(logits) are fp8. The residual path (qproj, MLP, layernorms) runs in
bf16 activations with f32 PSUM accumulation.

Engine balance (per core): PE issue slots are ~213ns per 512-col matmul
at 2.4GHz; scalar-engine EXP costs ~0.78ns/elem + ~320ns fixed per ACT.
The kernel runs ONE merged projection+attention pipeline: qproj first
(exps need qp and kp), then the lg->exp->att/den pipeline starts with
kproj h0 + the vp half for heads 0-3 done, and the remaining kproj/vproj
chunks stream into the PE as filler work between logits groups while the
scalar engine grinds exps — the PE never idles waiting for exp and the
scalar engine never idles waiting for projections. den uses fp8 DR pairs
into PSUM rows 0:32 with mones=16 folding the scale; its reciprocal is
partition-broadcast on the otherwise-idle GPSIMD. Attention drains also
pre-compute x1^2 (fp8) so LN1 stats matmuls start immediately. LN math
runs bf16 on the DVE (squares/casts on ACT), E[x^2] via fp8 DR pairs
(onesn8 = 2^-9 subnormal), the g/b affine via tensor_scalar — skipped
entirely when the host detects g=1,b=0. Output is stored bf16 and
upcast on the host.

Per-core dataflow (activations feature-major "T layout" [feat, row]):
  qpT = Wq @ qT            (bf16, dt-outer, double-buffered PSUM)
  vp  = v @ Wv.T * 16      (fp8 DoubleRow, natural [key, feat] layout)
  kpT = (Wk*16) @ kT       (fp8 DoubleRow, bf16 out at 16x scale)
  attention, 80 pipelined (head, chunk, key-tile-pair) steps:
      logitsT[kk,r] = kpT_h_tile.T @ xbfT_h   (bf16 matmul, PSUM f32)
      expT = Exp(logitsT / 512)               (ACT per pair, fp8 out)
      attT += vp_pair.T @ expT                (fp8 DoubleRow accumulate)
      denT += (16*mones_pair).T @ expT        (fp8 DoubleRow, rows 0:32)
  x1T = qpT + attT * recip(denT)   (gpsimd partition-broadcast of recip)
  out1 = LN(x1) [* g1 + b1]   via ones-matmul stats, bf16 DVE normalize
  x2T = out1 + Relu(Wout @ out1T + bout)      (bf16 matmul, ACT bias+relu)
  outT = LN(x2) [* g2 + b2] -> DRAM bf16 [P, feat-tile, row]; host
  reassembles and upcasts.
"""

from contextlib import ExitStack

import numpy as np
import ml_dtypes

import concourse.mybir as mybir
import concourse.tile as tile
from concourse import bacc
from concourse.bass_utils import run_bass_kernel_spmd

B, NQ, NK, D, H = 4, 2048, 2048, 1024, 8
DH = D // H            # 128, head dim
P = 128                # partitions
RQ = NQ // 2           # 1024 query rows per core
EPS = 1e-5

F32 = mybir.dt.float32
BF16 = mybir.dt.bfloat16
FP8 = mybir.dt.float8e4
BFNP = ml_dtypes.bfloat16
F8NP = ml_dtypes.float8_e4m3

KT = D // P            # 8 contraction tiles over features
DT = D // P            # 8 output-feature tiles (also heads)
RC = RQ // 512         # 2 row chunks of 512
DR = mybir.MatmulPerfMode.DoubleRow


def build_nc(kcap, plain_ln=False, debug=False):
    """kcap: padded (compacted) key count, multiple of 128.
    plain_ln: host-detected g1=g2=1, b1=b2=0 (skips the LN affines)."""
    assert kcap % 128 == 0
    KKT = kcap // 128          # key tiles
    KKTP = 2 * ((KKT + 1) // 2)    # mones key tiles padded to even
    kchunks = []
    o = 0
    while o < kcap:            # kproj output chunks (N dim), each <= 512
        n = min(512, kcap - o)
        kchunks.append((o, n))
        o += n

    # exp groups: key-tile pairs (DR-pair aligned for att/den); the scalar
    # engine is no longer the bottleneck once projections overlap attention
    GS = []
    t0 = 0
    while t0 < KKT:
        GS.append((t0, min(2, KKT - t0)))
        t0 += 2
    NG = len(GS)

    nc = bacc.Bacc("TRN2", target_bir_lowering=False, debug=debug)

    # all inputs pre-arranged to [P, tile, cols] on the host
    qT = nc.declare_dram_parameter("qT", [P, KT, RQ], BF16, isOutput=False)
    kT = nc.declare_dram_parameter("kT", [P, KT, kcap], FP8, isOutput=False)
    vT = nc.declare_dram_parameter("vT", [P, KT, kcap], FP8, isOutput=False)
    wqT = nc.declare_dram_parameter("wqT", [P, KT, D], BF16, isOutput=False)
    wkT = nc.declare_dram_parameter("wkT", [P, KT, D], FP8, isOutput=False)
    wvT = nc.declare_dram_parameter("wvT", [P, KT, D], FP8, isOutput=False)
    woT = nc.declare_dram_parameter("woT", [P, KT, D], BF16, isOutput=False)
    mones = nc.declare_dram_parameter("mones", [P, KKTP * 32], FP8, isOutput=False)
    vecs = nc.declare_dram_parameter("vecs", [P, 5, DT], F32, isOutput=False)
    outT = nc.declare_dram_parameter("outT", [P, DT, RQ], BF16, isOutput=True)

    Act = mybir.ActivationFunctionType

    with tile.TileContext(nc) as tc, ExitStack() as ctx:
        consts = ctx.enter_context(tc.tile_pool(name="consts", bufs=1))
        pool_qp = ctx.enter_context(tc.tile_pool(name="pool_qp", bufs=1))

        onesn = consts.tile([P, P], BF16)
        nc.vector.memset(onesn, 1.0 / D)
        # fp8 DR stationary for the E[x^2] matmuls: 2^-9 is the smallest
        # fp8e4 subnormal, so the sum over 1024 features is 2*E[x^2] and
        # the var chain halves it back
        onesn8 = consts.tile([P, 2, P], FP8)
        nc.vector.memset(onesn8, 2.0 ** -9)
        eps_sb = consts.tile([P, 1], F32)
        nc.vector.memset(eps_sb, EPS)
        # PE warm-up source: the DVFS ramp needs ~3us of continuous matmul
        # execution before reaching max clock, so dummy matmuls run during
        # the initial DMA wait (values irrelevant, result never read)
        warm_src = consts.tile([P, 512], BF16)
        nc.vector.memset(warm_src, 0.0)

        # persistent activations: qp -> x1 -> x2, bf16 (~0.4% rounding is
        # well inside the error budget and doubles DVE throughput)
        xbf_sb = pool_qp.tile([P, DT, RQ], BF16)
        # x1^2 (fp8), filled by the attention drains so LN1's E[x^2]
        # matmuls start immediately after the last drain
        sq1_sb = pool_qp.tile([P, DT, RQ], FP8)


        with (
            tc.tile_pool(name="pool_attn", bufs=1) as pool_attn,
            tc.tile_pool(name="pool_ain", bufs=1) as ain,
        ):
            kpT_sb = pool_attn.tile([P, H, kcap], BF16)  # per-head [dh, key], 16x
            vp_sb = pool_attn.tile([P, KKT, D], FP8)     # per key-tile [key, feat], 16x
            # ------- Merged pipeline: projections + attention -------
            # qproj runs first (exps need qp and kp), then the attention
            # lg->exp->att/den pipeline starts with kproj h0 + the vp half
            # used by heads 0-3 done; the REMAINING kproj heads and vp half
            # stream into the PE as filler work between logits groups while
            # the scalar engine grinds exps. All retained keys are unmasked;
            # zero-padded tail keys are excluded via zeroed vp rows and
            # zeroed den lhsT (mones, value 16 so den_ps = 16*den).
            with (
                tc.tile_pool(name="a_ps", bufs=2, space="PSUM") as a_ps,
                tc.tile_pool(name="att_ps", bufs=1, space="PSUM") as att_psp,
                tc.tile_pool(name="den_ps", bufs=1, space="PSUM") as den_psp,
                tc.tile_pool(name="lg_ps", bufs=2, space="PSUM") as lg_psp,
                tc.tile_pool(name="bsb", bufs=1) as bsb,
            ):
                # PE warm-up during the initial DMA wait (see warm_src)
                warm_ps = a_ps.tile([P, 512], F32, tag="aps")
                for _ in range(13):
                    nc.tensor.matmul(warm_ps, onesn, warm_src,
                                     start=True, stop=True)

                wq_sb = ain.tile([P, KT, D], BF16, tag="wq")
                qT_sb = ain.tile([P, KT, RQ], BF16, tag="qt")
                # kt-tile DMAs in first-use order: singles first so the
                # opening matmuls are gated on the smallest transfers
                for t0_, tn in ((0, 1), (1, 1), (2, 2), (4, 2), (6, 2)):
                    nc.sync.dma_start(out=wq_sb[:, t0_:t0_ + tn, :],
                                      in_=wqT[:, t0_:t0_ + tn, :])
                    nc.sync.dma_start(out=qT_sb[:, t0_:t0_ + tn, 0:512],
                                      in_=qT[:, t0_:t0_ + tn, 0:512])
                for t4 in range(0, KT, 4):
                    nc.sync.dma_start(out=qT_sb[:, t4:t4 + 4, 512:1024],
                                      in_=qT[:, t4:t4 + 4, 512:1024])
                kT_sb = ain.tile([P, KT, kcap], FP8, tag="kt")
                nc.sync.dma_start(out=kT_sb, in_=kT[:, :, :])
                wkT_sb = ain.tile([P, KT, D], FP8, tag="wk")
                nc.sync.dma_start(out=wkT_sb, in_=wkT[:, :, :])
                vT_sb = ain.tile([P, KT, kcap], FP8, tag="vv")
                nc.sync.dma_start(out=vT_sb, in_=vT[:, :, :])
                wvT_sb = ain.tile([P, KT, D], FP8, tag="wv")
                nc.sync.dma_start(out=wvT_sb, in_=wvT[:, :, :])
                mones_sb = consts.tile([P, KKTP, 32], FP8)
                nc.sync.dma_start(out=mones_sb, in_=mones[:, :])
                vecs_sb = consts.tile([P, 5, DT], F32)
                nc.sync.dma_start(out=vecs_sb, in_=vecs[:, :, :])
                g1_sb, b1_sb, g2_sb, b2_sb, bo_sb = (
                    vecs_sb[:, i, :] for i in range(5))

                # q projection, dt-outer (a_ps double-buffered; the first
                # matmul needs only the first wq/qT kt DMAs)
                for c in range(RC):
                    for dt_ in range(DT):
                        ps = a_ps.tile([P, 512], F32, tag="aps")
                        for kt in range(KT):
                            nc.tensor.matmul(
                                ps,
                                wq_sb[:, kt, dt_ * P:(dt_ + 1) * P],
                                qT_sb[:, kt, c * 512:(c + 1) * 512],
                                start=(kt == 0), stop=(kt == KT - 1),
                            )
                        nc.vector.tensor_copy(
                            xbf_sb[:, dt_, c * 512:(c + 1) * 512], ps)

                def emit_kproj(h, co, cn):
                    # one kproj chunk: kpT[h, co:co+cn] = ((Wk*16) @ k.T)
                    ps = a_ps.tile([P, 512], F32, tag="aps")
                    for tp in range(KT // 2):
                        nc.tensor.matmul(
                            ps[:, 0:cn],
                            wkT_sb[:, 2 * tp:2 * tp + 2, h * P:(h + 1) * P],
                            kT_sb[:, 2 * tp:2 * tp + 2, co:co + cn],
                            start=(tp == 0), stop=(tp == KT // 2 - 1),
                            perf_mode=DR,
                        )
                    nc.vector.tensor_copy(kpT_sb[:, h, co:co + cn],
                                          ps[:, 0:cn])

                def emit_vproj(kkt, dh):
                    # one vproj chunk: vp[kkt-tile, dh*512:...] * 16
                    ps = a_ps.tile([P, 512], F32, tag="aps")
                    for tp in range(KT // 2):
                        nc.tensor.matmul(
                            ps,
                            vT_sb[:, 2 * tp:2 * tp + 2, kkt * P:(kkt + 1) * P],
                            wvT_sb[:, 2 * tp:2 * tp + 2, dh * 512:(dh + 1) * 512],
                            start=(tp == 0), stop=(tp == KT // 2 - 1),
                            perf_mode=DR,
                        )
                    nc.vector.tensor_copy(vp_sb[:, kkt, dh * 512:(dh + 1) * 512],
                                          ps)

                # pre-loop: kproj head 0 and the vp D-half used by heads 0-3
                for (co, cn) in kchunks:
                    emit_kproj(0, co, cn)
                for kkt in range(KKT):
                    emit_vproj(kkt, 0)

                # filler units for the attention loop, in deadline order:
                # kproj for head h must land before head h's logits, the
                # second vp half before head 4's att matmuls
                fillers = []
                vp1 = [(kkt, 1) for kkt in range(KKT)]
                n3 = (len(vp1) + 2) // 3
                for hh in range(1, H):
                    fillers += [("kp", hh, co, cn) for (co, cn) in kchunks]
                    if hh <= 3:
                        tk = vp1[(hh - 1) * n3:hh * n3]
                        fillers += [("vp", kkt, dh, None) for (kkt, dh) in tk]

                steps = [(h, c, j)
                         for h in range(H) for c in range(RC) for j in range(NG)]
                exq = {}      # (h, c, j) -> ex tile
                cur = {}      # (h, c) -> (att_ps, den_ps)

                def emit_drain(h, c, att_ps, den_ps):
                    rs = slice(c * 512, (c + 1) * 512)
                    # copy att out of PSUM first: the next iteration's att
                    # accumulation reuses the bank, so freeing it fast keeps
                    # the in-order PE stream from stalling on this chain
                    attv = bsb.tile([P, 512], BF16, tag="attv", bufs=2)
                    nc.vector.tensor_copy(attv, att_ps)
                    # den_ps rows 0:32 hold 32 replicas of 16*den
                    rec32 = bsb.tile([32, 512], F32, tag="rec32", bufs=2)
                    nc.vector.reciprocal_approx_fast(rec32, den_ps[0:32, :])
                    rec = bsb.tile([P, 512], F32, tag="rec", bufs=2)
                    nc.gpsimd.partition_broadcast(rec, rec32[0:1, :])
                    att_n = bsb.tile([P, 512], F32, tag="attn", bufs=2)
                    nc.vector.tensor_mul(att_n, attv, rec)  # att/(16 den)
                    # x1 = qp + att  (in place over xbf)
                    nc.vector.tensor_add(xbf_sb[:, h, rs], xbf_sb[:, h, rs],
                                         att_n)
                    # x1^2 for LN1 stats while the DVE has slack
                    nc.vector.tensor_mul(sq1_sb[:, h, rs], xbf_sb[:, h, rs],
                                         xbf_sb[:, h, rs])

                def emit_lgexp(h, c, j):
                    rs = slice(c * 512, (c + 1) * 512)
                    g0, gn = GS[j]
                    lg_ps = lg_psp.tile([P, 2, 512], F32, tag="lg")
                    for t in range(gn):
                        nc.tensor.matmul(
                            lg_ps[:, t, :],
                            kpT_sb[:, h, (g0 + t) * P:(g0 + t + 1) * P],
                            xbf_sb[:, h, rs],
                            start=True, stop=True,
                        )
                    ex = bsb.tile([P, 2, 512], FP8, tag="ex", bufs=NG + 2)
                    # /512 = /16 (kp scale) /32 (sqrt(D))
                    nc.scalar.activation(ex[:, 0:gn, :], lg_ps[:, 0:gn, :],
                                         Act.Exp, scale=1.0 / 512.0)
                    exq[(h, c, j)] = ex

                def emit_attden(h, c, j):
                    if j == 0:
                        cur[(h, c)] = (
                            att_psp.tile([P, 512], F32, tag="att",
                                         name=f"att_{h}_{c}"),
                            den_psp.tile([P, 512], F32, tag="den",
                                         name=f"den_{h}_{c}"),
                        )
                    att_ps, den_ps = cur[(h, c)]
                    ex = exq.pop((h, c, j))
                    # one DR pair (or plain fp8 odd tail) each for att/den;
                    # single PSUM accumulation group per (h, c)
                    g0, gn = GS[j]
                    first = g0 == 0
                    last = g0 + gn == KKT
                    if gn == 2:
                        nc.tensor.matmul(
                            att_ps,
                            vp_sb[:, g0:g0 + 2, h * DH:(h + 1) * DH],
                            ex,
                            start=first, stop=last, perf_mode=DR,
                        )
                        nc.tensor.matmul(
                            den_ps[0:32, :],
                            mones_sb[:, g0:g0 + 2, :],
                            ex,
                            start=first, stop=last, perf_mode=DR,
                            skip_group_check=True,
                        )
                    else:
                        nc.tensor.matmul(
                            att_ps,
                            vp_sb[:, g0, h * DH:(h + 1) * DH],
                            ex[:, 0, :],
                            start=first, stop=last,
                        )
                        nc.tensor.matmul(
                            den_ps[0:32, :],
                            mones_sb[:, g0, :],
                            ex[:, 0, :],
                            start=first, stop=last,
                            skip_group_check=True,
                        )
                    if j == NG - 1:
                        emit_drain(h, c, att_ps, den_ps)
                        del cur[(h, c)]

                fi = 0
                for s in range(len(steps) + 2):
                    # one filler every other step spreads the projection
                    # slots across the whole loop (all deadlines still met)
                    if fi < len(fillers) and s % 2 == 0:
                        f = fillers[fi]
                        fi += 1
                        if f[0] == "kp":
                            emit_kproj(f[1], f[2], f[3])
                        else:
                            emit_vproj(f[1], f[2])
                    if s < len(steps):
                        emit_lgexp(*steps[s])
                    if s == len(steps):
                        # no act table holds both exp and sqrt: a dummy
                        # sqrt right after the last exp moves the ~1.3us
                        # table reload off phase C's critical entry chain
                        warm_sq = bsb.tile([P, 1], F32, tag="wsq", bufs=1)
                        nc.scalar.activation(warm_sq, eps_sb, Act.Sqrt)
                    if s >= 2:
                        emit_attden(*steps[s - 2])



        # ---------------- Phase C/D: LN1, MLP, LN2 ----------------
        # DVE does squares, normalize and the g/b affine (tensor_scalar with
        # per-partition AP scalars) on bf16 SBUF operands (fast DVE modes);
        # the scalar engine keeps only relu, mean^2 and sqrt.
        with (
            tc.tile_pool(name="late", bufs=1) as late,
            tc.tile_pool(name="csb", bufs=1) as csb,
            tc.tile_pool(name="c_ps", bufs=2, space="PSUM") as c_ps,
        ):
            woT_sb = late.tile([P, KT, D], BF16)
            nc.sync.dma_start(out=woT_sb, in_=woT[:, :, :])
            x1n_sb = late.tile([P, DT, RQ], BF16)
            rl_sb = late.tile([P, DT, RQ], BF16)
            ot_sb = late.tile([P, DT, RQ], BF16)

            cchunks = [(0, 512), (512, 512)]
            MUL = mybir.AluOpType.mult
            ADD = mybir.AluOpType.add

            def ln_stats_rc(rs, presq=None):
                """stats matmuls for one row-chunk of xbf: mean and E[x^2]
                PSUMs [P,512] (row vectors replicated across partitions).
                presq: pre-computed fp8 x^2 buffer (LN1); otherwise squares
                are computed here split between the scalar engine and DVE."""
                mean_ps = c_ps.tile([P, 512], F32, tag="mean")
                for kt in range(KT):
                    nc.tensor.matmul(
                        mean_ps, onesn, xbf_sb[:, kt, rs],
                        start=(kt == 0), stop=(kt == KT - 1),
                    )
                msq_ps = c_ps.tile([P, 512], F32, tag="msq")
                for pr in range(KT // 2):
                    if presq is not None:
                        sqp = presq[:, 2 * pr:2 * pr + 2, rs]
                    else:
                        sqt = csb.tile([P, 2, 512], FP8, tag="sqp", bufs=3)
                        nc.scalar.square(sqt[:, 0, :], xbf_sb[:, 2 * pr, rs])
                        nc.scalar.square(sqt[:, 1, :],
                                         xbf_sb[:, 2 * pr + 1, rs])
                        sqp = sqt
                    nc.tensor.matmul(
                        msq_ps, onesn8, sqp,
                        start=(pr == 0), stop=(pr == KT // 2 - 1),
                        perf_mode=DR,
                    )
                return mean_ps, msq_ps

            def ln_finish(mean_ps, msq_ps, mean_bf, rsg_bf, rs):
                """var/rstd chain; writes bf16 mean/rstd column slices of
                the full-width [P,RQ] stat tiles. The mean downcast goes
                first: mean_ps is ready before msq_ps, and the normalize
                subs only need the mean."""
                nc.scalar.copy(mean_bf[:, rs], mean_ps)
                musq = csb.tile([P, 512], F32, tag="musq", bufs=2)
                nc.scalar.square(musq, mean_ps)
                var = csb.tile([P, 512], F32, tag="var", bufs=2)
                # msq_ps holds 2*E[x^2] (see onesn8)
                nc.vector.scalar_tensor_tensor(
                    var, msq_ps, 0.5, musq,
                    mybir.AluOpType.mult, mybir.AluOpType.subtract)
                std = csb.tile([P, 512], F32, tag="std", bufs=2)
                nc.scalar.activation(std, var, Act.Sqrt,
                                     bias=eps_sb[:, :], scale=1.0)
                rsg = csb.tile([P, 512], F32, tag="rsg", bufs=2)
                nc.vector.reciprocal_approx_fast(rsg, std)
                nc.scalar.copy(rsg_bf[:, rs], rsg)

            # LN1: x1n = LN(x1) * g1 + b1; stats for both chunks first so
            # the var/rstd latency chains hide under the PE stats matmuls;
            # normalize runs chunk 0 first so the MLP z matmuls start early
            mean1_bf = csb.tile([P, RQ], BF16, tag="mean1b", bufs=1)
            rsg1_bf = csb.tile([P, RQ], BF16, tag="rsg1b", bufs=1)
            stats1 = [ln_stats_rc(slice(ro, ro + rn), presq=sq1_sb)
                      for (ro, rn) in cchunks]
            for ci, (ro, rn) in enumerate(cchunks):
                ln_finish(*stats1[ci], mean1_bf, rsg1_bf, slice(ro, ro + rn))
            for ci, (ro, rn) in enumerate(cchunks):
                rs = slice(ro, ro + rn)
                for kt in range(DT):
                    xc = csb.tile([P, 512], BF16, tag="xc1", bufs=4)
                    nc.vector.tensor_sub(xc, xbf_sb[:, kt, rs],
                                         mean1_bf[:, rs])
                    if plain_ln:
                        nc.vector.tensor_mul(x1n_sb[:, kt, rs], xc,
                                             rsg1_bf[:, rs])
                    else:
                        xh = csb.tile([P, 512], BF16, tag="xh1", bufs=3)
                        nc.vector.tensor_mul(xh, xc, rsg1_bf[:, rs])
                        nc.vector.tensor_scalar(
                            x1n_sb[:, kt, rs], xh, g1_sb[:, kt:kt + 1],
                            b1_sb[:, kt:kt + 1], MUL, ADD)

            # MLP: x2 = x1n + relu(Wout @ x1n.T + bout)  (x2 overwrites xbf);
            # LN2 stats per chunk right after its x2 so chain latency hides
            # under the other chunk's z matmuls
            def mlp_chunk(ro, rn):
                rs = slice(ro, ro + rn)
                for dt_ in range(DT):
                    z_ps = c_ps.tile([P, 512], F32, tag="z")
                    for kt in range(KT):
                        nc.tensor.matmul(
                            z_ps,
                            woT_sb[:, kt, dt_ * P:(dt_ + 1) * P],
                            x1n_sb[:, kt, rs],
                            start=(kt == 0), stop=(kt == KT - 1),
                        )
                    nc.scalar.activation(
                        rl_sb[:, dt_, rs], z_ps, Act.Relu,
                        bias=bo_sb[:, dt_:dt_ + 1], scale=1.0,
                    )
                    nc.vector.tensor_add(xbf_sb[:, dt_, rs],
                                         x1n_sb[:, dt_, rs],
                                         rl_sb[:, dt_, rs])

            mean2_bf = csb.tile([P, RQ], BF16, tag="mean2b", bufs=1)
            rsg2_bf = csb.tile([P, RQ], BF16, tag="rsg2b", bufs=1)

            def norm2_chunk(ro, rn, ci):
                # chunked LN2 normalize + store (bf16 out, host upcasts);
                # chunk 0 hides under chunk 1's stats, so its subs can use
                # gpsimd; the tail chunk splits store issues across queues
                rs = slice(ro, ro + rn)
                for kt in range(DT):
                    xc = csb.tile([P, 512], BF16, tag="xc2", bufs=4)
                    nc.vector.tensor_sub(xc, xbf_sb[:, kt, rs],
                                         mean2_bf[:, rs])
                    if plain_ln:
                        nc.vector.tensor_mul(ot_sb[:, kt, rs], xc,
                                             rsg2_bf[:, rs])
                    else:
                        xh = csb.tile([P, 512], BF16, tag="xh2", bufs=3)
                        nc.vector.tensor_mul(xh, xc, rsg2_bf[:, rs])
                        nc.vector.tensor_scalar(
                            ot_sb[:, kt, rs], xh, g2_sb[:, kt:kt + 1],
                            b2_sb[:, kt:kt + 1], MUL, ADD)
                    nc.sync.dma_start(out=outT[:, kt, rs],
                                      in_=ot_sb[:, kt, rs])

            # chunk-0 normalize runs under chunk-1's stats matmuls so the
            # final tail is only chunk 1's chain + normalize
            mlp_chunk(*cchunks[0])
            stats2_0 = ln_stats_rc(slice(0, 512))
            mlp_chunk(*cchunks[1])
            ln_finish(*stats2_0, mean2_bf, rsg2_bf, slice(0, 512))
            stats2_1 = ln_stats_rc(slice(512, 1024))
            norm2_chunk(*cchunks[0], 0)
            ln_finish(*stats2_1, mean2_bf, rsg2_bf, slice(512, 1024))
            norm2_chunk(*cchunks[1], 1)

    nc.compile()
    return nc


_NC_CACHE = {}


def get_nc(kcap=1280, plain_ln=False, debug=False):
    key = (kcap, plain_ln, debug)
    if key not in _NC_CACHE:
        _NC_CACHE[key] = build_nc(kcap, plain_ln=plain_ln, debug=debug)
    return _NC_CACHE[key]


def choose_plain_ln(g1, b1, g2, b2):
    return bool(
        np.all(np.asarray(g1) == 1.0) and np.all(np.asarray(b1) == 0.0)
        and np.all(np.asarray(g2) == 1.0) and np.all(np.asarray(b2) == 0.0))


def choose_kcap(mask):
    nkeep = int((~np.asarray(mask)).sum(axis=1).max())
    return max(256, -(-nkeep // 128) * 128)


def _tiles(a, cols):
    """[D, cols] -> [P, D//P, cols] partition-tiled layout."""
    return np.ascontiguousarray(
        np.asarray(a).reshape(KT, P, cols).transpose(1, 0, 2))


def shard_inputs(q, k, v, mask, Wq, Wk, Wv, Wout, bout, g1, b1, g2, b2,
                 kcap=None):
    q = np.asarray(q, dtype=np.float32)
    k = np.asarray(k, dtype=np.float32)
    v = np.asarray(v, dtype=np.float32)
    mask = np.asarray(mask)
    if kcap is None:
        kcap = choose_kcap(mask)
    KKT = kcap // 128
    KKTP = 2 * ((KKT + 1) // 2)    # mones tiles padded to even
    bfc = lambda a: np.ascontiguousarray(np.asarray(a, dtype=np.float32)).astype(BFNP)
    f8c = lambda a: np.ascontiguousarray(np.asarray(a, dtype=np.float32)).astype(F8NP)
    vec = lambda a: np.asarray(a, dtype=np.float32).reshape(DT, P).T

    vecs = np.stack([vec(g1), vec(b1), vec(g2), vec(b2), vec(bout)], axis=1)
    shared = {
        "wqT": bfc(_tiles(np.asarray(Wq, np.float32).T, D)),
        "wkT": f8c(_tiles(np.asarray(Wk, np.float32).T * 16.0, D)),
        "wvT": f8c(_tiles(np.asarray(Wv, np.float32).T * 16.0, D)),
        "woT": bfc(_tiles(np.asarray(Wout, np.float32).T, D)),
        "vecs": np.ascontiguousarray(vecs),
    }
    in_maps = []
    for bi in range(B):
        keep = np.where(~mask[bi])[0]
        nk = len(keep)
        kc = np.zeros((D, kcap), np.float32)
        vc = np.zeros((D, kcap), np.float32)
        kc[:, :nk] = k[bi][keep].T
        vc[:, :nk] = v[bi][keep].T
        mo = np.zeros((KKTP * P, 32), np.float32)  # [key, 32] -> [P, KKTP*32]
        mo[:nk] = 16.0    # den_ps = 16*den; its reciprocal undoes vp*16 too
        mo = mo.reshape(KKTP, P, 32).transpose(1, 0, 2).reshape(P, KKTP * 32)
        per_batch = {
            "kT": f8c(_tiles(kc, kcap)),
            "vT": f8c(_tiles(vc, kcap)),
            "mones": f8c(mo),
            **shared,
        }
        for half in range(2):
            rows = slice(half * RQ, (half + 1) * RQ)
            in_maps.append({
                "qT": bfc(_tiles(q[bi, rows].T, RQ)),
                **per_batch,
            })
    return in_maps


def assemble_output(results):
    out = np.empty((B, NQ, D), dtype=np.float32)
    for c in range(8):
        bi, half = divmod(c, 2)
        rows = slice(half * RQ, (half + 1) * RQ)
        # outT [P, DT, RQ] bf16 -> [RQ, DT*P] f32
        o = np.asarray(results[c]["outT"]).astype(np.float32)
        out[bi, rows, :] = o.transpose(2, 1, 0).reshape(RQ, D)
    return out


def kernel(**inputs):
    kcap = choose_kcap(inputs["mask"])
    plain = choose_plain_ln(inputs["g1"], inputs["b1"],
                            inputs["g2"], inputs["b2"])
    nc = get_nc(kcap, plain_ln=plain)
    in_maps = shard_inputs(**inputs, kcap=kcap)
    res = run_bass_kernel_spmd(nc, in_maps, core_ids=list(range(8)))
    return assemble_output(res.results)
